# revision 10
# baseline (speedup 1.0000x reference)
"""Trainium2 Bass kernel for nn_AttentionBlock (MLA-style attention + SwiGLU FFN).

Self-contained: takes FULL inputs, shards across 8 NeuronCores internally,
returns FULL output.

Sharding:
  Launch 1 (attention): tensor-parallel over heads (2 heads/core); each core
    computes its heads' partial attn_out @ W_O slice; host sums partials.
  Launch 2 (FFN): 2D sharding (4 token-quarters x 2 ffn-halves); host sums
    the two ffn-half partials per token quarter.
All matmuls run in bf16 with fp32 PSUM accumulation. Softmax/normalization
statistics are computed in fp32. Activations arrive pre-transposed (host does
the [m,D]->[D,m] transpose), so the device never transposes.
"""
import sys
sys.path.insert(0, '/opt/trn_rl_repo')

import math
import numpy as np
import ml_dtypes

from concourse import bass, bacc, mybir, tile
from concourse.bass_utils import run_bass_kernel_spmd
from wait_prune import prune_redundant_waits

BF = mybir.dt.bfloat16
F16 = mybir.dt.float16
F32 = mybir.dt.float32
AF = mybir.ActivationFunctionType

D = 2048
N_H = 16
D_H = 128
D_R = 64
FFN = 8192
THETA = 1000000.0
EPS = 1e-6
SCALE = 1.0 / math.sqrt(D_H + D_R)
NCORES = 8
P = 128
MB = 512


# --------------------------------------------------------------------------
# Launch 1: attention block, tensor-parallel over heads
# --------------------------------------------------------------------------
def build_attn(B, M, N, Dm, HPC, DH=D_H, DR=D_R):
    DC = Dm // P
    NT = N // P
    MT = M // P
    NBN = N // MB
    NBM = M // MB
    RD = HPC * DR
    HD = HPC * DH
    ln_scale_bias = float(math.log(SCALE))

    nc = bacc.Bacc()
    qT = nc.dram_tensor("qT", [B, Dm, M], BF, kind="ExternalInput")
    kvT = nc.dram_tensor("kvT", [B, Dm, N], BF, kind="ExternalInput")
    wq = nc.dram_tensor("wq", [Dm, HD], BF, kind="ExternalInput")
    wqr = nc.dram_tensor("wqr", [Dm, RD], BF, kind="ExternalInput")
    wk = nc.dram_tensor("wk", [Dm, HD], BF, kind="ExternalInput")
    wkr = nc.dram_tensor("wkr", [Dm, RD], BF, kind="ExternalInput")
    wv = nc.dram_tensor("wv", [Dm, HD], BF, kind="ExternalInput")
    wo = nc.dram_tensor("wo", [HD, Dm], BF, kind="ExternalInput")
    cos2T = nc.dram_tensor("cos2T", [RD, M], F16, kind="ExternalInput")
    sin2T = nc.dram_tensor("sin2T", [RD, M], F16, kind="ExternalInput")
    rot2T = nc.dram_tensor("rot2T", [RD, RD], BF, kind="ExternalInput")
    po = nc.dram_tensor("po", [B, M, Dm], F32, kind="ExternalOutput")

    with tile.TileContext(nc) as tc:
      with tc.tile_pool(name="const", bufs=1) as cp, \
           tc.tile_pool(name="dram", bufs=1, space="DRAM") as dramp:
        ones_bf = cp.tile([P, 1], BF, tag="ones")
        nc.vector.memset(ones_bf[:], 1.0)
        cosT_sb = cp.tile([RD, M], F16, tag="cos")
        sinT_sb = cp.tile([RD, M], F16, tag="sin")
        rot_sb = cp.tile([RD, RD], BF, tag="rot")
        eps_t = cp.tile([P, 1], F32, tag="eps")
        nc.vector.memset(eps_t[:], EPS)
        lnsc_t = cp.tile([P, 1], F32, tag="lnsc")
        nc.vector.memset(lnsc_t[:], ln_scale_bias)
        nc.sync.dma_start(out=cosT_sb[:], in_=cos2T[:])
        nc.sync.dma_start(out=sinT_sb[:], in_=sin2T[:])
        nc.sync.dma_start(out=rot_sb[:], in_=rot2T[:])

        for b in range(B):
          with tc.tile_pool(name=f"kq{b}", bufs=1) as kq:
            kt = [kq.tile([P, N], BF, tag=f"kt{h}", name=f"kt{h}") for h in range(HPC)]
            krt = kq.tile([RD, N], BF, tag="krt")
            vt = [kq.tile([P, HD], BF, tag=f"vt{i}", name=f"vt{i}") for i in range(NT)]
            qt = [kq.tile([P, M], BF, tag=f"qt{h}", name=f"qt{h}") for h in range(HPC)]
            qrt = kq.tile([RD, M], BF, tag="qrt")
            nkv_col = kq.tile([P, NT], F32, tag="nkvc")
            nkvV_col = kq.tile([P, NT], F32, tag="nkvvc")
            nq_bc = kq.tile([P, M], F32, tag="nqbc")

            # ================= KV side =================
            with tc.tile_pool(name=f"kvw{b}", bufs=1) as wp, \
                 tc.tile_pool(name=f"kvs{b}", bufs=2) as sp:
                wkt = [wp.tile([P, HD], BF, tag=f"wk{i}", name=f"wk{i}") for i in range(DC)]
                wkrt = [wp.tile([P, RD], BF, tag=f"wkr{i}", name=f"wkr{i}") for i in range(DC)]
                wvt = [wp.tile([P, HD], BF, tag=f"wv{i}", name=f"wv{i}") for i in range(DC)]
                kv_sb = [wp.tile([P, N], BF, tag=f"akv{i}", name=f"akv{i}") for i in range(DC)]
                for dc in range(DC):
                    nc.sync.dma_start(out=wkt[dc][:], in_=wk[dc * P:(dc + 1) * P, :])
                    nc.sync.dma_start(out=wkrt[dc][:], in_=wkr[dc * P:(dc + 1) * P, :])
                    nc.sync.dma_start(out=wvt[dc][:], in_=wv[dc * P:(dc + 1) * P, :])
                    nc.sync.dma_start(out=kv_sb[dc][:], in_=kvT[b, dc * P:(dc + 1) * P, :])

                # --- rms stats: sum_d(x^2) via Square + ones-matmul ---
                with tc.tile_pool(name=f"kvn{b}", bufs=1, space="PSUM") as pn:
                    sumsq = [pn.tile([1, MB], F32, tag=f"ss{nb}", name=f"ss{nb}") for nb in range(NBN)]
                    for dc in range(DC):
                        sq = sp.tile([P, N], BF, tag="sq")
                        nc.scalar.activation(sq[:], kv_sb[dc][:], AF.Square)
                        for nb in range(NBN):
                            nc.tensor.matmul(sumsq[nb][:], ones_bf[:],
                                             sq[:, nb * MB:(nb + 1) * MB],
                                             start=(dc == 0), stop=(dc == DC - 1))
                    ln_row = sp.tile([1, N], F32, tag="lnrow")
                    for nb in range(NBN):
                        nc.scalar.activation(ln_row[0:1, nb * MB:(nb + 1) * MB],
                                             sumsq[nb][:], AF.Ln,
                                             scale=1.0 / Dm, bias=eps_t[0:1, :])
                drow = dramp.tile([1, N], F32, tag=f"dkv{b}")
                nc.sync.dma_start(out=drow[:], in_=ln_row[:])
                lncol = sp.tile([P, NT], F32, tag="lncol")
                nc.sync.dma_start(out=lncol[:],
                                  in_=drow[:].rearrange("a (t p) -> (a p) t", p=P))
                nc.scalar.activation(nkv_col[:], lncol[:], AF.Exp,
                                     scale=-0.5, bias=lnsc_t[:])
                nc.scalar.activation(nkvV_col[:], lncol[:], AF.Exp, scale=-0.5)

                pp_cm = tc.tile_pool(name=f"kvp{b}", bufs=2, space="PSUM")
                pp = pp_cm.__enter__()
                # --- K_C^T projection ---
                for h in range(HPC):
                    for nb in range(NBN):
                        ps = pp.tile([P, MB], F32, tag="proj")
                        for dc in range(DC):
                            nc.tensor.matmul(ps[:], wkt[dc][:, h * DH:(h + 1) * DH],
                                             kv_sb[dc][:, nb * MB:(nb + 1) * MB],
                                             start=(dc == 0), stop=(dc == DC - 1))
                        nc.scalar.copy(kt[h][:, nb * MB:(nb + 1) * MB], ps[:])
                # --- K_R^T projection (heads stacked on partitions) ---
                krt_raw = sp.tile([RD, N], BF, tag="krraw")
                for nb in range(NBN):
                    ps = pp.tile([RD, MB], F32, tag="projr")
                    for dc in range(DC):
                        nc.tensor.matmul(ps[:], wkrt[dc][:],
                                         kv_sb[dc][:, nb * MB:(nb + 1) * MB],
                                         start=(dc == 0), stop=(dc == DC - 1))
                    nc.scalar.copy(krt_raw[:, nb * MB:(nb + 1) * MB], ps[:])
                # --- V projection (activations stationary, nkv-scaled evac) ---
                for nt in range(NT):
                    ps = pp.tile([P, HD], F32, tag="projv")
                    for dc in range(DC):
                        nc.tensor.matmul(ps[:], kv_sb[dc][:, nt * P:(nt + 1) * P],
                                         wvt[dc][:],
                                         start=(dc == 0), stop=(dc == DC - 1))
                    nc.vector.tensor_scalar_mul(vt[nt][:], ps[:],
                                                nkvV_col[:, nt:nt + 1])
                # --- rope K ---
                for nb in range(NBN):
                    nbs = slice(nb * MB, (nb + 1) * MB)
                    rps = pp.tile([RD, MB], F32, tag="rot")
                    nc.tensor.matmul(rps[:], rot_sb[:], krt_raw[:, nbs],
                                     start=True, stop=True)
                    c_t = sp.tile([RD, MB], BF, tag="ropec")
                    nc.vector.tensor_mul(c_t[:], krt_raw[:, nbs], cosT_sb[:, nbs])
                    s_t = sp.tile([RD, MB], BF, tag="ropes")
                    nc.vector.tensor_mul(s_t[:], rps[:], sinT_sb[:, nbs])
                    nc.vector.tensor_add(krt[:, nbs], c_t[:], s_t[:])
                pp_cm.__exit__(None, None, None)

            # ================= Q side =================
            with tc.tile_pool(name=f"qw{b}", bufs=1) as wp, \
                 tc.tile_pool(name=f"qs{b}", bufs=2) as sp:
                wqt = [wp.tile([P, HD], BF, tag=f"wq{i}", name=f"wq{i}") for i in range(DC)]
                wqrt = [wp.tile([P, RD], BF, tag=f"wqr{i}", name=f"wqr{i}") for i in range(DC)]
                q_sb = [wp.tile([P, M], BF, tag=f"aq{i}", name=f"aq{i}") for i in range(DC)]
                for dc in range(DC):
                    nc.sync.dma_start(out=wqt[dc][:], in_=wq[dc * P:(dc + 1) * P, :])
                    nc.sync.dma_start(out=wqrt[dc][:], in_=wqr[dc * P:(dc + 1) * P, :])
                    nc.sync.dma_start(out=q_sb[dc][:], in_=qT[b, dc * P:(dc + 1) * P, :])

                with tc.tile_pool(name=f"qn{b}", bufs=1, space="PSUM") as pn:
                    sumsq = [pn.tile([1, MB], F32, tag=f"ss{nb}", name=f"ss{nb}") for nb in range(NBM)]
                    for dc in range(DC):
                        sq = sp.tile([P, M], BF, tag="sq")
                        nc.scalar.activation(sq[:], q_sb[dc][:], AF.Square)
                        for nb in range(NBM):
                            nc.tensor.matmul(sumsq[nb][:], ones_bf[:],
                                             sq[:, nb * MB:(nb + 1) * MB],
                                             start=(dc == 0), stop=(dc == DC - 1))
                    ln_row = sp.tile([1, M], F32, tag="lnrow")
                    for nb in range(NBM):
                        nc.scalar.activation(ln_row[0:1, nb * MB:(nb + 1) * MB],
                                             sumsq[nb][:], AF.Ln,
                                             scale=1.0 / Dm, bias=eps_t[0:1, :])
                    nq_row = sp.tile([1, M], F32, tag="nqrow")
                    nc.scalar.activation(nq_row[:], ln_row[:], AF.Exp, scale=-0.5)
                drow = dramp.tile([1, M], F32, tag=f"dq{b}")
                nc.sync.dma_start(out=drow[:], in_=nq_row[:])
                nc.sync.dma_start(out=nq_bc[:], in_=drow[:].to_broadcast((P, M)))
                pp_cm = tc.tile_pool(name=f"qp{b}", bufs=2, space="PSUM")
                pp = pp_cm.__enter__()

                for h in range(HPC):
                    for nb in range(NBM):
                        nbs = slice(nb * MB, (nb + 1) * MB)
                        ps = pp.tile([P, MB], F32, tag="proj")
                        for dc in range(DC):
                            nc.tensor.matmul(ps[:], wqt[dc][:, h * DH:(h + 1) * DH],
                                             q_sb[dc][:, nbs],
                                             start=(dc == 0), stop=(dc == DC - 1))
                        nc.vector.tensor_mul(qt[h][:, nbs], ps[:], nq_bc[:, nbs])
                qrt_raw = sp.tile([RD, M], BF, tag="qrraw")
                for nb in range(NBM):
                    nbs = slice(nb * MB, (nb + 1) * MB)
                    ps = pp.tile([RD, MB], F32, tag="projr")
                    for dc in range(DC):
                        nc.tensor.matmul(ps[:], wqrt[dc][:], q_sb[dc][:, nbs],
                                         start=(dc == 0), stop=(dc == DC - 1))
                    nc.vector.tensor_mul(qrt_raw[:, nbs], ps[:], nq_bc[:RD, nbs])
                for nb in range(NBM):
                    nbs = slice(nb * MB, (nb + 1) * MB)
                    rps = pp.tile([RD, MB], F32, tag="rot")
                    nc.tensor.matmul(rps[:], rot_sb[:], qrt_raw[:, nbs],
                                     start=True, stop=True)
                    c_t = sp.tile([RD, MB], BF, tag="ropec")
                    nc.vector.tensor_mul(c_t[:], qrt_raw[:, nbs], cosT_sb[:, nbs])
                    s_t = sp.tile([RD, MB], BF, tag="ropes")
                    nc.vector.tensor_mul(s_t[:], rps[:], sinT_sb[:, nbs])
                    nc.vector.tensor_add(qrt[:, nbs], c_t[:], s_t[:])
                pp_cm.__exit__(None, None, None)

            # ================= attention + W_O =================
            with tc.tile_pool(name=f"at{b}", bufs=1) as ap, \
                 tc.tile_pool(name=f"ap{b}", bufs=2, space="PSUM") as pp, \
                 tc.tile_pool(name=f"ae{b}", bufs=2 * NT + 2) as ep, \
                 tc.tile_pool(name=f"as{b}", bufs=2) as sp:
                ut = [ap.tile([P, M], BF, tag=f"ut{h}", name=f"ut{h}") for h in range(HPC)]
                wo_sb = [ap.tile([P, Dm], BF, tag=f"wo{h}", name=f"wo{h}") for h in range(HPC)]
                for h in range(HPC):
                    nc.sync.dma_start(out=wo_sb[h][:], in_=wo[h * DH:(h + 1) * DH, :])
                drs = dramp.tile([1, M], F32, tag=f"drs{b}")

                for h in range(HPC):
                    for mb in range(NBM):
                        mbs = slice(mb * MB, (mb + 1) * MB)
                        u_ps = pp.tile([P, MB], F32, tag="u")
                        sum_ps = pp.tile([1, MB], F32, tag="sums")
                        for nt in range(NT):
                            s_ps = pp.tile([P, MB], F32, tag="s")
                            nc.tensor.matmul(s_ps[:], kt[h][:, nt * P:(nt + 1) * P],
                                             qt[h][:, mbs], start=True, stop=False)
                            nc.tensor.matmul(
                                s_ps[:],
                                krt[h * DR:(h + 1) * DR, nt * P:(nt + 1) * P],
                                qrt[h * DR:(h + 1) * DR, mbs],
                                start=False, stop=True)
                            et = ep.tile([P, MB], BF, tag="et")
                            nc.scalar.activation(et[:], s_ps[:], AF.Exp,
                                                 scale=nkv_col[:, nt:nt + 1])
                            nc.tensor.matmul(u_ps[:], vt[nt][:, h * DH:(h + 1) * DH],
                                             et[:], start=(nt == 0),
                                             stop=(nt == NT - 1))
                            nc.tensor.matmul(sum_ps[:], ones_bf[:], et[:],
                                             start=(nt == 0), stop=(nt == NT - 1))
                        rs_row = sp.tile([1, MB], F32, tag="rs")
                        nc.vector.reciprocal(rs_row[:], sum_ps[:])
                        nc.sync.dma_start(out=drs[0:1, mbs], in_=rs_row[:])
                        rsb = sp.tile([P, MB], F32, tag="rsb")
                        nc.sync.dma_start(out=rsb[:],
                                          in_=drs[0:1, mbs].to_broadcast((P, MB)))
                        nc.vector.tensor_mul(ut[h][:, mbs], u_ps[:], rsb[:])

                for mt in range(MT):
                    po_sb = sp.tile([P, Dm], F32, tag="po")
                    for ocb in range(Dm // MB):
                        w_ps = pp.tile([P, MB], F32, tag="wops")
                        for h in range(HPC):
                            nc.tensor.matmul(w_ps[:], ut[h][:, mt * P:(mt + 1) * P],
                                             wo_sb[h][:, ocb * MB:(ocb + 1) * MB],
                                             start=(h == 0), stop=(h == HPC - 1))
                        nc.scalar.copy(po_sb[:, ocb * MB:(ocb + 1) * MB], w_ps[:])
                    nc.sync.dma_start(out=po[b, mt * P:(mt + 1) * P, :], in_=po_sb[:])
    prune_redundant_waits(nc, verbose=True)
    nc.compile()
    return nc


# --------------------------------------------------------------------------
# Launch 2: FFN, token-quarter x ffn-half sharding
# --------------------------------------------------------------------------
def build_ffn(TOK, Dm, FH, act_fn=None):
    DC = Dm // P
    FC = FH // P
    NBM = TOK // MB
    MTT = TOK // P

    nc = bacc.Bacc()
    xnT = nc.dram_tensor("xnT", [Dm, TOK], BF, kind="ExternalInput")
    wg = nc.dram_tensor("wg", [Dm, FH], BF, kind="ExternalInput")
    wu = nc.dram_tensor("wu", [Dm, FH], BF, kind="ExternalInput")
    wd = nc.dram_tensor("wd", [FH, Dm], BF, kind="ExternalInput")
    fo = nc.dram_tensor("fo", [TOK, Dm], F32, kind="ExternalOutput")

    with tile.TileContext(nc) as tc:
      with tc.tile_pool(name="xp", bufs=1) as xp, \
           tc.tile_pool(name="hp", bufs=1) as hp:
        xn_sb = [xp.tile([P, TOK], BF, tag=f"xn{i}", name=f"xn{i}") for i in range(DC)]
        for dc in range(DC):
            nc.sync.dma_start(out=xn_sb[dc][:], in_=xnT[dc * P:(dc + 1) * P, :])
        ht = [hp.tile([P, TOK], BF, tag=f"h{i}", name=f"h{i}") for i in range(FC)]

        with tc.tile_pool(name="gw", bufs=4) as gw, \
             tc.tile_pool(name="gp", bufs=2, space="PSUM") as gps, \
             tc.tile_pool(name="gs", bufs=3) as gsp:
            for fc in range(FC):
                g_ps = gps.tile([P, TOK], F32, tag="g")
                u_ps = gps.tile([P, TOK], F32, tag="u")
                for dc in range(DC):
                    wgt = gw.tile([P, P], BF, tag="wg")
                    wut = gw.tile([P, P], BF, tag="wu")
                    nc.sync.dma_start(
                        out=wgt[:], in_=wg[dc * P:(dc + 1) * P, fc * P:(fc + 1) * P])
                    nc.sync.dma_start(
                        out=wut[:], in_=wu[dc * P:(dc + 1) * P, fc * P:(fc + 1) * P])
                    for nb in range(NBM):
                        mbs = slice(nb * MB, (nb + 1) * MB)
                        nc.tensor.matmul(g_ps[:, mbs], wgt[:], xn_sb[dc][:, mbs],
                                         start=(dc == 0), stop=(dc == DC - 1))
                        nc.tensor.matmul(u_ps[:, mbs], wut[:], xn_sb[dc][:, mbs],
                                         start=(dc == 0), stop=(dc == DC - 1))
                hs = gsp.tile([P, TOK], BF, tag="hs")
                nc.scalar.activation(hs[:], g_ps[:],
                                 AF.Silu if act_fn is None else act_fn)
                nc.vector.tensor_mul(ht[fc][:], hs[:], u_ps[:])

        with tc.tile_pool(name="dw", bufs=2) as dw, \
             tc.tile_pool(name="dp", bufs=4, space="PSUM") as dps, \
             tc.tile_pool(name="ds", bufs=3) as dsp:
            for ocb in range(Dm // MB):
                ocs = slice(ocb * MB, (ocb + 1) * MB)
                wdt = [dw.tile([P, MB], BF, tag=f"wd{fc}", name=f"wd{fc}") for fc in range(FC)]
                for fc in range(FC):
                    nc.sync.dma_start(out=wdt[fc][:], in_=wd[fc * P:(fc + 1) * P, ocs])
                for mt in range(MTT):
                    d_ps = dps.tile([P, MB], F32, tag="d")
                    for fc in range(FC):
                        nc.tensor.matmul(d_ps[:], ht[fc][:, mt * P:(mt + 1) * P],
                                         wdt[fc][:],
                                         start=(fc == 0), stop=(fc == FC - 1))
                    o_sb = dsp.tile([P, MB], F32, tag="o")
                    nc.scalar.copy(o_sb[:], d_ps[:])
                    nc.sync.dma_start(out=fo[mt * P:(mt + 1) * P, ocs], in_=o_sb[:])
    prune_redundant_waits(nc, verbose=True)
    nc.compile()
    return nc


# --------------------------------------------------------------------------
# Host orchestration
# --------------------------------------------------------------------------
_prog_cache = {}


def _get(key, builder, *args):
    if key not in _prog_cache:
        _prog_cache[key] = builder(*args)
    return _prog_cache[key]


def _bf(x):
    return np.ascontiguousarray(np.asarray(x, dtype=np.float32)).astype(
        ml_dtypes.bfloat16)


def _rope_tables(S, dim):
    freqs = 1.0 / (THETA ** (np.arange(0, dim, 2, dtype=np.float32) / dim))
    f = np.arange(S, dtype=np.float32)[:, None] * freqs[None, :]
    cos = np.repeat(np.cos(f), 2, axis=-1).astype(np.float32)
    sin = np.repeat(np.sin(f), 2, axis=-1).astype(np.float32)
    return cos, sin


def _rot_lhsT(dim):
    rt = np.zeros((dim, dim), np.float32)
    for i in range(dim // 2):
        rt[2 * i + 1, 2 * i] = -1.0
        rt[2 * i, 2 * i + 1] = 1.0
    return rt


_last_exec_ns = []


def _run(nc, in_maps, trace=False):
    res = run_bass_kernel_spmd(nc, in_maps, list(range(len(in_maps))), trace=trace)
    _last_exec_ns.append(res.exec_time_ns)
    return res


def kernel(query, key_value, g_q, g_kv, g_ffn, w_qc, w_kc, w_qr, w_kr, w_v,
           w_o, w_gate, w_up, w_down, _trace=False):
    query = np.asarray(query, np.float32)
    key_value = np.asarray(key_value, np.float32)
    Bq, Mq, _ = query.shape
    Nq = key_value.shape[1]
    HPC = N_H // NCORES

    g_q = np.asarray(g_q, np.float32)[:, None]
    g_kv = np.asarray(g_kv, np.float32)[:, None]
    g_ffn = np.asarray(g_ffn, np.float32)[:, None]
    wqc = np.asarray(w_qc, np.float32) * g_q
    wqr_f = np.asarray(w_qr, np.float32) * g_q
    wkc = np.asarray(w_kc, np.float32) * g_kv
    wkr_f = np.asarray(w_kr, np.float32) * g_kv
    wv_f = np.asarray(w_v, np.float32) * g_kv
    wo_f = np.asarray(w_o, np.float32)
    wgate = np.asarray(w_gate, np.float32) * g_ffn
    wup = np.asarray(w_up, np.float32) * g_ffn
    wdown = np.asarray(w_down, np.float32)

    qT = _bf(query.transpose(0, 2, 1))
    kvT = _bf(key_value.transpose(0, 2, 1))
    cos, sin = _rope_tables(max(Mq, Nq), D_R)
    cos2T = np.ascontiguousarray(np.vstack([cos[:Mq].T] * HPC)).astype(np.float16)
    sin2T = np.ascontiguousarray(np.vstack([sin[:Mq].T] * HPC)).astype(np.float16)
    rot2T = _bf(np.kron(np.eye(HPC, dtype=np.float32), _rot_lhsT(D_R)))

    del _last_exec_ns[:]
    nc1 = _get(("attn", Bq, Mq, Nq, D, HPC), build_attn, Bq, Mq, Nq, D, HPC)
    in_maps = []
    for c in range(NCORES):
        hs = slice(c * HPC * D_H, (c + 1) * HPC * D_H)
        rs = slice(c * HPC * D_R, (c + 1) * HPC * D_R)
        in_maps.append({
            "qT": qT, "kvT": kvT,
            "wq": _bf(wqc[:, hs]), "wqr": _bf(wqr_f[:, rs]),
            "wk": _bf(wkc[:, hs]), "wkr": _bf(wkr_f[:, rs]),
            "wv": _bf(wv_f[:, hs]), "wo": _bf(wo_f[hs, :]),
            "cos2T": cos2T, "sin2T": sin2T, "rot2T": rot2T,
        })
    res1 = _run(nc1, in_maps, trace=_trace)
    attn = np.zeros((Bq, Mq, D), np.float32)
    for r in res1.results:
        attn += r["po"]

    x = query + attn
    xf = x.reshape(Bq * Mq, D)
    n = 1.0 / np.sqrt((xf * xf).mean(axis=-1, keepdims=True) + EPS)
    xn = xf * n
    TQ = 4
    FHALF = FFN // 2
    tok = Bq * Mq // TQ
    xnT_sh = [_bf(xn[t * tok:(t + 1) * tok, :].T) for t in range(TQ)]
    wg_h = [_bf(wgate[:, :FHALF]), _bf(wgate[:, FHALF:])]
    wu_h = [_bf(wup[:, :FHALF]), _bf(wup[:, FHALF:])]
    wd_h = [_bf(wdown[:FHALF, :]), _bf(wdown[FHALF:, :])]

    nc2 = _get(("ffn", tok, D, FHALF), build_ffn, tok, D, FHALF)
    in_maps2 = []
    for c in range(NCORES):
        tq, fh = c % TQ, c // TQ
        in_maps2.append({"xnT": xnT_sh[tq], "wg": wg_h[fh], "wu": wu_h[fh],
                         "wd": wd_h[fh]})
    res2 = _run(nc2, in_maps2, trace=_trace)

    ffn_out = np.zeros((Bq * Mq, D), np.float32)
    for c in range(NCORES):
        tq = c % TQ
        ffn_out[tq * tok:(tq + 1) * tok, :] += res2.results[c]["fo"]

    y = x + ffn_out.reshape(Bq, Mq, D)
    return y
